# revision 4
# baseline (speedup 1.0000x reference)
"""Trainium2 Bass kernel for nn_ContrastiveLoss (SimCLR-style NT-Xent loss).

Math: z = concat(f1, f2) [2B, D]; zn = z / ||z||_row;
logits = zn @ zn.T / T; labels[i] = i mod B;
loss = mean_i(logsumexp(logits[i, :]) - logits[i, label_i]).

Distribution: data-parallel over rows of z across 8 NeuronCores. Each
core computes its 1024-row block of logits against all 8192 columns
(bf16 GEMM on the PE), with the softmax statistics fused on the fly:
exp(2*cos) with per-instruction free-dim accumulation on the Scalar
engine, so the full 8192x8192 logits matrix is never materialized.
The target logit is computed separately as an elementwise row-dot
(t_r = 2 * zn_r . zn_label(r)), so no gather is needed. Row norms are
computed on-device from the transposed operand via Square (ACT) +
ones-matmul partition reduction (PE). The host only does layout
(concat/slice/permute/transpose), sharding, and the final 8-way sum.
"""

import numpy as np

import concourse.bass as bass
import concourse.mybir as mybir
import concourse.tile as tile
from concourse.bass_utils import run_bass_kernel_spmd
from concourse.vector_clock import ScopedClock

F32 = mybir.dt.float32
BF16 = mybir.dt.bfloat16
AF = mybir.ActivationFunctionType
ALU = mybir.AluOpType

B = 4096
D = 512
N2 = 2 * B          # 8192 rows of z
NCORES = 8
ROWS = N2 // NCORES  # 1024 rows per core
MT = ROWS // 128     # 8 m-tiles per core
KT = D // 128        # 4 k-tiles
CHUNK = 2048         # GEMM column chunk (4 PSUM banks)
NCH = N2 // CHUNK    # 4 GEMM column chunks
SCHUNK = 1024        # normalize/scale column chunk
NSC = N2 // SCHUNK   # 8 scale chunks
TEMP_INV = 2.0       # 1 / temperature


# ---------------------------------------------------------------------------
# Patches for this toolchain build:
# 1) walrus CoreV2/V3 codegen only accepts ONE sync wait per instruction;
#    Tile attaches several (tail drain, multi-dep DMAs). Split extras onto
#    standalone EventSemaphore instructions placed immediately before the
#    overloaded instruction (same engine, same basic block) — blocking at
#    engine-issue time is strictly more conservative and deadlock-free
#    because Tile's per-engine streams preserve global dependency order.
# ---------------------------------------------------------------------------
_MAX_WAITS = 1
_patched = False


def _patched_drain_and_barrier(self, tick_clock, wait_clock):
    nc = self.nc
    drain_inst = nc.sync.drain()
    wait_clock.add_sem_waits(
        drain_inst.ins, ScopedClock({None: tick_clock.global_clock})
    )
    si = drain_inst.ins.sync_info
    if si is not None and si.on_wait and len(si.on_wait) > _MAX_WAITS:
        waits = list(si.on_wait)
        si.on_wait = waits[:_MAX_WAITS]
        for i in range(_MAX_WAITS, len(waits), _MAX_WAITS):
            extra = nc.sync.drain()
            extra.ins.sync_info = mybir.SyncInfo(
                on_wait=waits[i : i + _MAX_WAITS], on_update=[]
            )
    nc.all_engine_barrier()
    assert self.sems is not None
    popped = nc._tile_sem_poison_stack.pop()
    assert popped is self._sem_poison
    nc.clear_and_free_semaphores(list(self.sems.allocated().values()))
    nc.all_engine_barrier()


def _apply_patches():
    global _patched
    if _patched:
        return
    tile.TileContext._drain_and_barrier = _patched_drain_and_barrier
    _patched = True


def _split_waits(nc):
    n = 0
    for fn in nc.m.functions:
        for bb in fn.blocks:
            insts = bb.instructions
            if not any(
                i.sync_info
                and i.sync_info.on_wait
                and len(i.sync_info.on_wait) > _MAX_WAITS
                for i in insts
            ):
                continue
            out = []
            for inst in insts:
                si = inst.sync_info
                if si and si.on_wait and len(si.on_wait) > _MAX_WAITS:
                    waits = list(si.on_wait)
                    for w in waits[:-_MAX_WAITS]:
                        n += 1
                        ev = mybir.InstEventSemaphore(
                            name=f"WSPLIT-{n}", ins=[], outs=[]
                        )
                        ev.engine = inst.engine
                        ev.sync_info = mybir.SyncInfo(on_wait=[w], on_update=[])
                        out.append(ev)
                    si.on_wait = waits[-_MAX_WAITS:]
                out.append(inst)
            bb.instructions = out
    return n


# ---------------------------------------------------------------------------
# Device kernel (identical program on all 8 cores; per-core data differs)
# ---------------------------------------------------------------------------
def _build_nc():
    _apply_patches()
    nc = bass.Bass()

    # zt:   [D, N2]  f32 — z rows (own-rows-first per core) transposed
    # zown: [ROWS,D] f32 — this core's own rows (row-major)
    # zlab: [ROWS,D] f32 — label rows for this core's rows (row-major)
    zt = nc.declare_dram_parameter("zt", [D, N2], F32, isOutput=False)
    zown = nc.declare_dram_parameter("zown", [ROWS, D], F32, isOutput=False)
    zlab = nc.declare_dram_parameter("zlab", [ROWS, D], F32, isOutput=False)
    out = nc.declare_dram_parameter("out", [128, MT], F32, isOutput=True)

    inv_dram = nc.dram_tensor("inv_bounce", [1, N2], F32)

    with tile.TileContext(nc) as tc:
        with (
            tc.tile_pool(name="persist", bufs=1) as persist,
            tc.tile_pool(name="ztst", bufs=2) as ztst_pool,
            tc.tile_pool(name="sq", bufs=2) as sq_pool,
            tc.tile_pool(name="invb", bufs=2) as invb_pool,
            tc.tile_pool(name="small", bufs=2) as small_pool,
            tc.tile_pool(name="rows", bufs=3) as rows_pool,
            tc.tile_pool(name="psum", bufs=2, space="PSUM") as psum_pool,
        ):
            # persistent tensors
            znT = [
                persist.tile([128, N2], BF16, tag=f"znT{k}", name=f"znT{k}") for k in range(KT)
            ]
            ones = persist.tile([128, 1], BF16, tag="ones")
            nc.vector.memset(ones, 1.0)
            acc = persist.tile([128, MT, NCH], F32, tag="acc")
            sso = persist.tile([128, MT], F32, tag="sso")
            ssl = persist.tile([128, MT], F32, tag="ssl")
            dotr = persist.tile([128, MT], F32, tag="dotr")

            # ---- per column-chunk: load zt, norms^2 via Square+ones-matmul,
            #      inv-norm broadcast, scale to bf16 znT -------------------
            for cc in range(NSC):
                cs = slice(cc * SCHUNK, (cc + 1) * SCHUNK)
                ps = psum_pool.tile([128, CHUNK], F32)
                ztst = {}
                for kt in range(KT):
                    st = ztst_pool.tile([128, SCHUNK], F32, tag=f"zt{kt}", name=f"zt{kt}")
                    nc.sync.dma_start(
                        out=st, in_=zt.ap()[kt * 128 : (kt + 1) * 128, cs]
                    )
                    ztst[kt] = st
                    sq = sq_pool.tile([128, SCHUNK], BF16, tag="sq")
                    nc.scalar.activation(out=sq, in_=st, func=AF.Square)
                    for n in range(SCHUNK // 512):
                        nc.tensor.matmul(
                            ps[0:1, n * 512 : (n + 1) * 512],
                            ones,
                            sq[:, n * 512 : (n + 1) * 512],
                            start=(kt == 0),
                            stop=(kt == KT - 1),
                        )
                # sqrt + reciprocal on the single-partition norms^2 row
                nrm = small_pool.tile([1, SCHUNK], F32, tag="nrm")
                nc.scalar.activation(out=nrm, in_=ps[0:1, 0:SCHUNK], func=AF.Sqrt)
                inv1 = small_pool.tile([1, SCHUNK], F32, tag="inv1")
                nc.vector.reciprocal(out=inv1, in_=nrm)
                # broadcast to 128 partitions via DRAM round-trip
                nc.sync.dma_start(out=inv_dram.ap()[0:1, cs], in_=inv1)
                bsrc = bass.AP(
                    tensor=inv_dram.ap().tensor,
                    offset=cc * SCHUNK,
                    ap=[[0, 128], [1, SCHUNK]],
                )
                invb = invb_pool.tile([128, SCHUNK], F32, tag="invb")
                nc.sync.dma_start(out=invb, in_=bsrc)
                # scale: znT[kt][:, chunk] = zt_chunk * inv (bf16 out)
                for kt in range(KT):
                    nc.vector.tensor_mul(znT[kt][:, cs], ztst[kt], invb)

            # ---- own/label rows: norms + target logit t = 2*zn_o.zn_l ----
            for m in range(MT):
                ms = slice(m * 128, (m + 1) * 128)
                ow = rows_pool.tile([128, D], F32, tag="own")
                nc.sync.dma_start(out=ow, in_=zown.ap()[ms, :])
                lb = rows_pool.tile([128, D], F32, tag="lab")
                nc.sync.dma_start(out=lb, in_=zlab.ap()[ms, :])
                sco = rows_pool.tile([128, D], BF16, tag="sqsc")
                nc.scalar.activation(
                    out=sco, in_=ow, func=AF.Square, accum_out=sso[:, m : m + 1]
                )
                scl = rows_pool.tile([128, D], BF16, tag="sqsc")
                nc.scalar.activation(
                    out=scl, in_=lb, func=AF.Square, accum_out=ssl[:, m : m + 1]
                )
                prod = rows_pool.tile([128, D], F32, tag="prod")
                nc.vector.tensor_mul(prod, ow, lb)
                nc.vector.tensor_reduce(
                    out=dotr[:, m : m + 1], in_=prod,
                    axis=mybir.AxisListType.X, op=ALU.add,
                )
            nrmo = persist.tile([128, MT], F32, tag="nrmo")
            nc.scalar.activation(out=nrmo, in_=sso, func=AF.Sqrt)
            invo = persist.tile([128, MT], F32, tag="invo")
            nc.vector.reciprocal(out=invo, in_=nrmo)
            nrml = persist.tile([128, MT], F32, tag="nrml")
            nc.scalar.activation(out=nrml, in_=ssl, func=AF.Sqrt)
            invl = persist.tile([128, MT], F32, tag="invl")
            nc.vector.reciprocal(out=invl, in_=nrml)
            t2 = persist.tile([128, MT], F32, tag="t2")
            nc.vector.tensor_mul(t2, dotr, invo)
            nc.vector.tensor_mul(t2, t2, invl)
            nc.vector.tensor_scalar_mul(t2, t2, TEMP_INV)

            # ---- GEMM + fused exp/accumulate --------------------------------
            # logits chunk = znT_own(m).T @ znT_all(chunk); exp(2x) with
            # free-dim accumulation, written back in-place to PSUM.
            for nb in range(NCH):
                for m in range(MT):
                    ps = psum_pool.tile([128, CHUNK], F32)
                    for kt in range(KT):
                        lhsT = znT[kt][:, m * 128 : (m + 1) * 128]
                        for n in range(CHUNK // 512):
                            col = nb * CHUNK + n * 512
                            nc.tensor.matmul(
                                ps[:, n * 512 : (n + 1) * 512],
                                lhsT,
                                znT[kt][:, col : col + 512],
                                start=(kt == 0),
                                stop=(kt == KT - 1),
                            )
                    nc.scalar.activation(
                        out=ps, in_=ps, func=AF.Exp, scale=TEMP_INV,
                        accum_out=acc[:, m, nb : nb + 1],
                    )

            # ---- finalize: lse = ln(sum exp), partials = lse - t ---------
            ssum = persist.tile([128, MT], F32, tag="ssum")
            nc.vector.tensor_reduce(
                out=ssum, in_=acc, axis=mybir.AxisListType.X, op=ALU.add
            )
            lse = persist.tile([128, MT], F32, tag="lse")
            nc.scalar.activation(out=lse, in_=ssum, func=AF.Ln)
            diff = persist.tile([128, MT], F32, tag="diff")
            nc.vector.tensor_sub(diff, lse, t2)
            nc.sync.dma_start(out=out.ap(), in_=diff)

    _split_waits(nc)
    return nc


_nc_cache = None


def _get_nc():
    global _nc_cache
    if _nc_cache is None:
        _nc_cache = _build_nc()
    return _nc_cache


# ---------------------------------------------------------------------------
# Host wrapper: shard, run SPMD on cores 0-7, reduce
# ---------------------------------------------------------------------------
def kernel(features_1, features_2, _trace=False):
    f1 = np.ascontiguousarray(np.asarray(features_1, dtype=np.float32))
    f2 = np.ascontiguousarray(np.asarray(features_2, dtype=np.float32))
    assert f1.shape == (B, D) and f2.shape == (B, D)
    z = np.concatenate([f1, f2], axis=0)  # [N2, D]

    in_maps = []
    allrows = np.arange(N2)
    for c in range(NCORES):
        own_lo = c * ROWS
        lab_lo = (c % (B // ROWS)) * ROWS
        # device column order: own rows first, then the rest
        rest = np.concatenate([allrows[:own_lo], allrows[own_lo + ROWS :]])
        R = np.concatenate([allrows[own_lo : own_lo + ROWS], rest])
        zR = z[R]
        in_maps.append(
            {
                "zt": np.ascontiguousarray(zR.T),
                "zown": np.ascontiguousarray(z[own_lo : own_lo + ROWS]),
                "zlab": np.ascontiguousarray(z[lab_lo : lab_lo + ROWS]),
            }
        )

    nc = _get_nc()
    res = run_bass_kernel_spmd(
        nc, in_maps, core_ids=list(range(NCORES)), trace=_trace
    )
    total = np.float64(0.0)
    for c in range(NCORES):
        total += res.results[c]["out"].astype(np.float64).sum()
    loss = np.float32(total / N2)
    if _trace:
        return loss, res
    return loss
